# revision 8
# baseline (speedup 1.0000x reference)
"""Trainium2 Bass kernel for nn_MultiHeadAttention_85864986182183.

Reference computation (B=4, S=4096, E=1024, D=64, H=16 identical heads):
    q = x @ Wq + bq; k = x @ Wk + bk; v = x @ Wv + bv          [B,S,D]
    attn = softmax(q @ k^T / sqrt(D))                           [B,S,S]
    ctx = attn @ v                                              [B,S,D]
    out = tile(ctx, H) @ Wo + bo                                [B,S,E]

Algebraic folds used here:
  * tile(ctx,H) @ Wo == ctx @ Wo_eff  with Wo_eff[d,:] = sum_h Wo[h*D+d,:]
  * softmax denominators come for free from a ones-column appended to V
  * out rows are scaled by 1/den AFTER the output projection; appending the
    denominator row to ctx^T and bo as the matching Wo_eff row makes the
    +bo exact under that scaling (den * (1/den) * bo == bo).

Sharding: core c handles batch b=c//2, query half h=c%2 (2048 queries, all
4096 keys; K/V projection work is duplicated across the pair - cheaper than
exchanging K/V between cores).

Layouts on device (per core):
  xT  [E(+1), S]   streamed in 8 column blocks of 512
  qT  [64, 2048]   (d on partitions)   kT [64, 4096]
  vaug[128, 32, 65] k-chunk-major V with ones column
  scores^T tiles [128k, 512q] -> exp -> P^T tiles -> ctx^T accumulation
  ctx^T_aug [65, 512] -> output projection lhsT, recip scaling at the end
"""

import os
import numpy as np

import concourse.bass as bass
import concourse.mybir as mybir
import concourse.tile as tile
from concourse import bacc
from concourse.bass_utils import run_bass_kernel_spmd

f32 = mybir.dt.float32
f32r = mybir.dt.float32r

B, S, E, D, H = 4, 4096, 1024, 64, 16
NCORES = 8
SQ = S // 2            # queries per core
NSB = S // 512         # 8 s-blocks
NKC = S // 128         # 32 k-chunks
NQB = SQ // 512        # 4 q-blocks per core
SCALE = 1.0 / np.sqrt(D)

_PROGRAM_CACHE = {}


def _build_program(with_bias: bool):
    """Returns the Bass program (shared by all cores, SPMD)."""
    EA = E + 1 if with_bias else E           # augmented contraction for q/k/v bias
    NEC = EA // 128 + (1 if EA % 128 else 0)  # e-chunks (8 or 9; last may be 1 row)

    nc = bacc.Bacc("TRN2", target_bir_lowering=False, debug=False)

    xt_d = nc.declare_dram_parameter("xt", [EA, S], f32, isOutput=False)
    wq_d = nc.declare_dram_parameter("wq", [EA, D], f32, isOutput=False)
    wk_d = nc.declare_dram_parameter("wk", [EA, D], f32, isOutput=False)
    wv_d = nc.declare_dram_parameter("wv", [EA, D], f32, isOutput=False)
    wo_d = nc.declare_dram_parameter("wo", [D + 1, E], f32, isOutput=False)
    out_d = nc.declare_dram_parameter("out", [SQ, E], f32, isOutput=True)

    # Cores differ only in which half of xT holds their queries: the host
    # rolls xT columns for odd cores so the query half is ALWAYS [0, 2048).
    # The roll permutes key order identically in kT and vaug, and softmax
    # over keys is permutation-invariant, so outputs are unchanged.

    with tile.TileContext(nc) as tc:
        with (
            tc.tile_pool(name="const", bufs=1) as constp,
            tc.tile_pool(name="wsb", bufs=1) as wp,
            tc.tile_pool(name="persist", bufs=1) as pp,
            tc.tile_pool(name="xts", bufs=3) as xtp,
            tc.tile_pool(name="vtmp", bufs=2) as vtmpp,
            tc.tile_pool(name="ptp", bufs=3) as ptp,
            tc.tile_pool(name="outp", bufs=3) as outp,
            tc.tile_pool(name="smallp", bufs=4) as smallp,
        ):
            # ---- constants / weights ----
            ident = constp.tile([64, 64], f32)
            nc.gpsimd.memset(ident[:], 0.0)
            from concourse.masks import make_identity
            make_identity(nc, ident[:], nomemset=True)
            ident1 = constp.tile([1, 1], f32)
            nc.vector.memset(ident1[:], 1.0)

            wq_sb = wp.tile([128, NEC, D], f32r)
            wk_sb = wp.tile([128, NEC, D], f32r)
            wv_sb = wp.tile([128, NEC, D], f32r)
            for w_sb, w_d in ((wq_sb, wq_d), (wk_sb, wk_d), (wv_sb, wv_d)):
                w_r = w_d[: 8 * 128, :].rearrange("(c p) d -> p c d", p=128)
                nc.gpsimd.dma_start(w_sb[:, :8, :], w_r)
                if NEC == 9:  # bias row -> partition 0 of chunk 8
                    nc.gpsimd.dma_start(w_sb[:1, 8, :], w_d[E : E + 1, :])
            wo_sb = wp.tile([D + 1, E], f32r)
            nc.gpsimd.dma_start(wo_sb[:], wo_d[:])

            qt = pp.tile([64, SQ], f32r)
            kt = pp.tile([64, S], f32r)
            vaug = pp.tile([128, NKC, 65], f32r)
            ones_sb = constp.tile([128, NKC, 1], f32)
            nc.vector.memset(ones_sb[:], 1.0)
            nc.vector.tensor_copy(vaug[:, :, 64:65], ones_sb[:])

            xt_r = xt_d[: 8 * 128, :].rearrange("(c p) s -> p c s", p=128)

            # ================= Phase A: projections =================
            with (
                tc.tile_pool(name="projps", bufs=3, space="PSUM") as projps,
                tc.tile_pool(name="vchps", bufs=2, space="PSUM") as vchps,
            ):
                for i in range(NSB):
                    sb = slice(i * 512, (i + 1) * 512)
                    xt_t = xtp.tile([128, NEC, 512], f32r, tag="xt", name=f"xt{i}")
                    nc.gpsimd.dma_start(xt_t[:, :8, :], xt_r[:, :, sb])
                    if NEC == 9:
                        nc.gpsimd.dma_start(xt_t[:1, 8, :], xt_d[E : E + 1, sb])

                    def proj(w_sb, name):
                        ps = projps.tile([64, 512], f32, tag="proj", name=name)
                        for c in range(NEC):
                            kpart = 128 if c < 8 else 1
                            nc.tensor.matmul(
                                ps[:],
                                w_sb[:kpart, c, :],
                                xt_t[:kpart, c, :],
                                start=(c == 0),
                                stop=(c == NEC - 1),
                            )
                        return ps

                    kt_ps = proj(wk_sb, f"ktps{i}")
                    nc.vector.tensor_copy(kt[:, sb], kt_ps[:])
                    if i < 4:  # query half lives in columns [0, 2048)
                        qt_ps = proj(wq_sb, f"qtps{i}")
                        nc.vector.tensor_copy(qt[:, sb], qt_ps[:])
                    vt_ps = proj(wv_sb, f"vtps{i}")
                    vt_sb = vtmpp.tile([64, 512], f32, tag="vt", name=f"vt{i}")
                    nc.vector.tensor_copy(vt_sb[:], vt_ps[:])
                    for t in range(4):
                        kc = i * 4 + t
                        v_ps = vchps.tile([128, 64], f32, tag="vch", name=f"vch{kc}")
                        nc.tensor.transpose(
                            v_ps[:], vt_sb[:, t * 128 : (t + 1) * 128], ident[:]
                        )
                        nc.vector.tensor_copy(vaug[:, kc, 0:64], v_ps[:])

            # ================= Phase B: attention =================
            with (
                tc.tile_pool(name="stps", bufs=2, space="PSUM") as stps,
                tc.tile_pool(name="ctxps", bufs=1, space="PSUM") as ctxps,
                tc.tile_pool(name="rcps", bufs=1, space="PSUM") as rcpsp,
                tc.tile_pool(name="opps", bufs=2, space="PSUM") as opps,
            ):
                for qb in range(NQB):
                    qs = slice(qb * 512, (qb + 1) * 512)
                    ctx_ps = ctxps.tile([65, 512], f32, tag="ctx", name=f"ctx{qb}")
                    for kp in range(NKC // 2):
                        st_ps = stps.tile([128, 1024], f32, tag="st", name=f"st{qb}_{kp}")
                        pt = ptp.tile([128, 1024], f32r, tag="pt", name=f"pt{qb}_{kp}")
                        for h2 in range(2):
                            kc = kp * 2 + h2
                            nc.tensor.matmul(
                                st_ps[:, h2 * 512 : (h2 + 1) * 512],
                                kt[:, kc * 128 : (kc + 1) * 128],
                                qt[:, qs],
                                start=True,
                                stop=True,
                            )
                        nc.scalar.activation(
                            pt[:], st_ps[:], mybir.ActivationFunctionType.Exp, scale=SCALE
                        )
                        for h2 in range(2):
                            kc = kp * 2 + h2
                            nc.tensor.matmul(
                                ctx_ps[:],
                                vaug[:, kc, :],
                                pt[:, h2 * 512 : (h2 + 1) * 512],
                                start=(kc == 0),
                                stop=(kc == NKC - 1),
                            )
                    ctx_sb = smallp.tile([65, 512], f32r, tag="ctxsb", name=f"ctxsb{qb}")
                    nc.vector.tensor_copy(ctx_sb[:], ctx_ps[:])
                    recip_row = smallp.tile([1, 512], f32, tag="rrow", name=f"rrow{qb}")
                    nc.vector.reciprocal(recip_row[:], ctx_sb[64:65, :])
                    rc_ps = rcpsp.tile([128, 4], f32, tag="rcps", name=f"rcps{qb}")
                    for t in range(4):
                        nc.tensor.transpose(
                            rc_ps[:, t : t + 1],
                            recip_row[:, t * 128 : (t + 1) * 128],
                            ident1[:],
                        )
                    recip_col = smallp.tile([128, 4], f32, tag="rcol", name=f"rcol{qb}")
                    nc.vector.tensor_copy(recip_col[:], rc_ps[:])
                    for t in range(4):
                        out_sb = outp.tile([128, E], f32, tag="out", name=f"out{qb}_{t}")
                        for h2 in range(2):
                            op_ps = opps.tile(
                                [128, 512], f32, tag="op", name=f"op{qb}_{t}_{h2}"
                            )
                            nc.tensor.matmul(
                                op_ps[:],
                                ctx_sb[:, t * 128 : (t + 1) * 128],
                                wo_sb[:, h2 * 512 : (h2 + 1) * 512],
                                start=True,
                                stop=True,
                            )
                            nc.vector.tensor_scalar_mul(
                                out_sb[:, h2 * 512 : (h2 + 1) * 512],
                                op_ps[:],
                                recip_col[:, t : t + 1],
                            )
                        r0 = qb * 512 + t * 128
                        nc.sync.dma_start(out_d[r0 : r0 + 128, :], out_sb[:])

    nc.compile()
    return nc


def kernel(x, Wq, bq, Wk, bk, Wv, bv, Wo, bo, _trace=False):
    x = np.asarray(x, dtype=np.float32)
    Wq, bq = np.asarray(Wq, np.float32), np.asarray(bq, np.float32)
    Wk, bk = np.asarray(Wk, np.float32), np.asarray(bk, np.float32)
    Wv, bv = np.asarray(Wv, np.float32), np.asarray(bv, np.float32)
    Wo, bo = np.asarray(Wo, np.float32), np.asarray(bo, np.float32)

    with_bias = bool(np.any(bq) or np.any(bk) or np.any(bv))
    key = with_bias
    if key not in _PROGRAM_CACHE:
        _PROGRAM_CACHE[key] = _build_program(with_bias)
    nc = _PROGRAM_CACHE[key]

    # Host-side weight prep (tiny).
    wo_eff = Wo.reshape(H, D, E).astype(np.float64).sum(axis=0)
    wo_aug = np.concatenate([wo_eff, bo[None, :].astype(np.float64)], axis=0)
    wo_aug = np.ascontiguousarray(wo_aug, dtype=np.float32)
    if with_bias:
        wq_a = np.concatenate([Wq, bq[None, :]], 0)
        wk_a = np.concatenate([Wk, bk[None, :]], 0)
        wv_a = np.concatenate([Wv, bv[None, :]], 0)
    else:
        wq_a, wk_a, wv_a = Wq, Wk, Wv

    in_maps = []
    for c in range(NCORES):
        b, h = c // 2, c % 2
        xt = np.ascontiguousarray(x[b].T)  # [E, S]
        if h == 1:
            # roll so this core's query half occupies columns [0, 2048);
            # key order is permuted identically in kT and vaug -> softmax
            # result for each query is unchanged.
            xt = np.ascontiguousarray(np.roll(xt, -SQ, axis=1))
        if with_bias:
            xt = np.concatenate([xt, np.ones((1, S), np.float32)], 0)
        in_maps.append({"xt": xt, "wq": wq_a, "wk": wk_a, "wv": wv_a, "wo": wo_aug})

    res = run_bass_kernel_spmd(
        nc, in_maps, list(range(NCORES)), trace=_trace
    )
    out = np.empty((B, S, E), dtype=np.float32)
    for c in range(NCORES):
        b, h = c // 2, c % 2
        out[b, h * SQ : (h + 1) * SQ, :] = res.results[c]["out"]
    if _trace:
        kernel._last_exec_time_ns = res.exec_time_ns
        kernel._last_results = res
    return out
